# revision 36
# baseline (speedup 1.0000x reference)
"""LIF (leaky integrate-and-fire) forward kernel for Trainium2, 8 NeuronCores.

Recurrence (per element of [B, N], serial over T):
    v_t = DECAY * w_{t-1} + x_t          (REST = 0, w = post-reset membrane)
    s_t = (v_t > THRESHOLD)
    w_t = v_t * (v_t <= THRESHOLD)

The kernel is DVE-throughput-bound (DVE is the only engine that runs fused
two-tensor fp32 elementwise at full rate), so the v-update for E=1536 of the
2048 columns is offloaded to the otherwise-idle PE as fp32 diagonal matmuls
accumulated in PSUM (all 8 banks are available since the output streams as
fp8 signs):

  E cols:  psv = (DECAY*I) @ w_E  (+)  I @ x_E     [PE, 6 matmuls/step]
           st_E = Sign(psv - THR) (fp8, per bank)  [ScalarE, PSUM read]
           w_E  = (st_E <= 0) * psv (per bank)     [DVE, one PSUM operand]
  D cols:  v = (w_D * DECAY) + x_D ; w_D = (v<=THR)*v ; st_D = Sign(v-THR)
           [classic DVE pair + ScalarE]

Per-step engine budget: DVE ~3.2us, PE ~2.6us, ACT ~2.2us (vs 4.6us on DVE
alone). PE's fp32 matmul rounds within 1 ULP of the reference on ~8% of
elements; a spike flips only if |v-THR| < 1 ULP, i.e. ~1 bit in the whole
run -- far inside the 2e-2 relative-error budget. Host decodes (out > 0).

Sharding: batch dim (128) split 16 rows/core across 8 cores; per-core,
per-step slab is a contiguous 1 MiB block viewed as [128 partitions, 2048].
"""

import numpy as np

import concourse.bacc as bacc
import concourse.mybir as mybir
from concourse.tile import TileContext
from concourse.bass_utils import run_bass_kernel_spmd

T, B, N = 32, 128, 16384
N_CORES = 8
B_SH = B // N_CORES          # 16 batch rows per core
S = B_SH * N                 # 262144 elements per core per time step
P = 128                      # SBUF partitions
F = S // P                   # 2048 free-dim elements
E = 1024                     # columns whose v-update runs on PE (2 PSUM banks)
DECAY = 0.2
THR = 0.3

TRACE = False                # set True (e.g. from test.py) to capture a profile

_BUILT = {}


def _build_nc():
    nc = bacc.Bacc("TRN2", debug=False, num_devices=N_CORES)
    x = nc.dram_tensor("x", [T, S], mybir.dt.float32, kind="ExternalInput").ap()
    y = nc.dram_tensor("y", [T, S], mybir.dt.float8e4, kind="ExternalOutput").ap()
    xr = x.rearrange("t (p f) -> t p f", p=P)
    yr = y.rearrange("t (p f) -> t p f", p=P)

    f32 = mybir.dt.float32
    Alu = mybir.AluOpType
    Act = mybir.ActivationFunctionType

    H = F // 2
    with TileContext(nc) as tc:
        with (
            tc.tile_pool(name="state", bufs=1) as state_pool,
            tc.tile_pool(name="xin", bufs=10) as xin_pool,
            tc.tile_pool(name="vtmp", bufs=4) as v_pool,
            tc.tile_pool(name="sout", bufs=8) as s_pool,
            # one PSUM pool per 512-col bank: keeps the matmul->sign->w
            # dependency per-bank (a single wide tile makes every sign wait
            # for all six matmuls of the step)
            tc.tile_pool(name="psv0", bufs=2, space="PSUM") as psv0_pool,
            tc.tile_pool(name="psv1", bufs=2, space="PSUM") as psv1_pool,
        ):
            psv_pools = (psv0_pool, psv1_pool)
            negthr = nc.alloc_sbuf_tensor("const_negthr", [P, 1], f32).ap()
            nc.gpsimd.memset(negthr, -THR)

            # fp32 diagonal weights DECAY*I and I off one Pool iota p - f
            wtmp = nc.alloc_sbuf_tensor("wk_iota", [P, 128], f32).ap()
            nc.gpsimd.iota(
                wtmp, pattern=[[-1, 128]], base=0, channel_multiplier=1,
                allow_small_or_imprecise_dtypes=True,
            )
            w_dec = nc.alloc_sbuf_tensor("w_decay_eye", [P, 128], f32).ap()
            w_eye = nc.alloc_sbuf_tensor("w_eye", [P, 128], f32).ap()
            nc.vector.tensor_scalar(
                out=w_dec, in0=wtmp, scalar1=0.0, scalar2=DECAY,
                op0=Alu.is_equal, op1=Alu.mult,
            )
            nc.vector.tensor_scalar(
                out=w_eye, in0=wtmp, scalar1=0.0, scalar2=1.0,
                op0=Alu.is_equal, op1=Alu.mult,
            )

            w = state_pool.tile([P, F], f32)

            # Loads are emitted LOOKAHEAD iterations early. The sync queue is
            # in-order, so emitting load(t+1) after out(t) would stall every
            # load behind the previous step's spike store (which waits on that
            # step's sign) -- defeating the prefetch pool entirely.
            LOOKAHEAD = 8
            xts = {}

            def load(tt):
                xt = xin_pool.tile([P, F], f32)
                if tt == 0:
                    # quarter the first load so compute starts on 256 KiB
                    for j in range(0, F, 512):
                        nc.sync.dma_start(
                            out=xt[:, j:j + 512], in_=xr[tt][:, j:j + 512]
                        )
                elif tt == 1:
                    # t=1 rides the ACT HWDGE queue in parallel with the t=0
                    # load; its dispatch-stall on ACT sits inside ACT's idle
                    # head window, and the chain starts ~4us sooner
                    nc.scalar.dma_start(out=xt[:], in_=xr[tt])
                else:
                    nc.sync.dma_start(out=xt[:], in_=xr[tt])
                xts[tt] = xt

            for tt in range(min(LOOKAHEAD, T)):
                load(tt)

            for t in range(T):
                if t + LOOKAHEAD < T:
                    load(t + LOOKAHEAD)
                xt = xts.pop(t)

                st = s_pool.tile([P, F], mybir.dt.float8e4)
                if t == 0:
                    # w_{-1}=0 so v_0 = x_0: read x directly, per quarter
                    for j in range(0, F, 512):
                        nc.vector.scalar_tensor_tensor(
                            out=w[:, j:j + 512], in0=xt[:, j:j + 512], scalar=THR,
                            in1=xt[:, j:j + 512], op0=Alu.is_le, op1=Alu.mult,
                        )
                        nc.scalar.activation(
                            st[:, j:j + 512], xt[:, j:j + 512], Act.Sign, bias=negthr
                        )
                        nc.sync.dma_start(
                            out=yr[t][:, j:j + 512], in_=st[:, j:j + 512]
                        )
                else:
                    last = t == T - 1
                    # E cols: v on PE -> PSUM (one bank tile per 512 cols)
                    psvs = []
                    for b, j in enumerate(range(0, E, 512)):
                        pb = psv_pools[b].tile([P, 512], f32)
                        psvs.append(pb)
                        nc.tensor.matmul(
                            out=pb[:], lhsT=w_dec, rhs=w[:, j:j + 512],
                            start=True, stop=False,
                        )
                        nc.tensor.matmul(
                            out=pb[:], lhsT=w_eye, rhs=xt[:, j:j + 512],
                            start=False, stop=True,
                        )
                    # D cols: classic DVE pair (w-update dead at last step)
                    v = v_pool.tile([P, F - E], f32)
                    nc.vector.scalar_tensor_tensor(
                        out=v[:], in0=w[:, E:], scalar=DECAY, in1=xt[:, E:],
                        op0=Alu.mult, op1=Alu.add,
                    )
                    if not last:
                        nc.vector.scalar_tensor_tensor(
                            out=w[:, E:], in0=v[:], scalar=THR, in1=v[:],
                            op0=Alu.is_le, op1=Alu.mult,
                        )
                    # E cols per bank: sign from PSUM, then w_E from the sign
                    # (one PSUM operand; st<=0 is exactly v<=THR)
                    for b, j in enumerate(range(0, E, 512)):
                        pb = psvs[b]
                        nc.scalar.activation(
                            st[:, j:j + 512], pb[:], Act.Sign, bias=negthr,
                        )
                        if not last:
                            nc.vector.scalar_tensor_tensor(
                                out=w[:, j:j + 512], in0=st[:, j:j + 512],
                                scalar=0.0, in1=pb[:],
                                op0=Alu.is_le, op1=Alu.mult,
                            )
                        if last:
                            # drain the final stores on both HWDGE queues
                            # (ACT has no work left to stall at this point)
                            q = nc.sync if b % 2 == 0 else nc.scalar
                            q.dma_start(
                                out=yr[t][:, j:j + 512], in_=st[:, j:j + 512]
                            )
                    nc.scalar.activation(st[:, E:], v[:], Act.Sign, bias=negthr)
                    if last:
                        nc.sync.dma_start(out=yr[t][:, E:E + 512], in_=st[:, E:E + 512])
                        nc.scalar.dma_start(out=yr[t][:, E + 512:], in_=st[:, E + 512:])
                    else:
                        nc.sync.dma_start(out=yr[t], in_=st[:])
    nc.compile()
    return nc


LAST_RESULTS = None


def kernel(tx):
    global LAST_RESULTS
    tx = np.asarray(tx)
    assert tx.shape == (T, B, N) and tx.dtype == np.float32

    if "nc" not in _BUILT:
        _BUILT["nc"] = _build_nc()
    nc = _BUILT["nc"]

    in_maps = [
        {"x": np.ascontiguousarray(tx[:, c * B_SH:(c + 1) * B_SH, :]).reshape(T, S)}
        for c in range(N_CORES)
    ]
    res = run_bass_kernel_spmd(nc, in_maps, core_ids=list(range(N_CORES)), trace=TRACE)
    LAST_RESULTS = res

    out = np.empty((T, B, N), dtype=np.float32)
    for c in range(N_CORES):
        sgn = np.asarray(res.results[c]["y"]).reshape(T, B_SH, N)
        out[:, c * B_SH:(c + 1) * B_SH, :] = (sgn > 0).astype(np.float32)
    return out
